# revision 1
# baseline (speedup 1.0000x reference)
"""ACSF descriptor kernel for 8 Trainium2 NeuronCores (Bass/Tile).

Scheme
------
Output rows (atoms) are sharded across the 8 cores (6250 atoms each); no
all-reduce is needed.  The host does integer-only topology preprocessing:
  * drops triplets failing the integer mask id3_ba > id3_ca,
  * computes each item's destination atom row and species column-slot
    (from the idx_mapping tables),
  * routes items to the owning core, sorts them by 16-atom destination
    block, and pads each block's item list to a whole number of
    128-item tiles (identical tile counts on every core, so one SPMD
    program serves all 8 cores).
The device does all floating-point math: cutoff functions
fc(r) = 0.5 cos(pi r / rc) + 0.5 (via the Sin LUT), exponentials, the
law-of-cosines R_bc, the lambda/zeta power terms, and the scatter-add.

Scatter-add on device: for each 128-item tile the DVE builds a one-hot
matrix over M "virtual rows" (16 atoms x 3 pair-slots for G4 = 48;
16 atoms x 2 species = 32 for G2) by comparing the per-item virtual
index against an iota row; TensorE then computes
one_hot^T @ values -> PSUM, accumulating over the tiles of each
16-atom block.  ScalarE copies each finished PSUM block to SBUF and the
result is DMA'd out as [M, NBLK*width]; the host reorders columns into
the reference layout.
"""

import math
from contextlib import ExitStack

import numpy as np

P = 128          # partitions / items per tile
N_ATOMS = 50000
N_CORES = 8
APC = N_ATOMS // N_CORES      # atoms per core (6250)
BLK = 16                      # atoms per PSUM block
NBLK = (APC + BLK - 1) // BLK # 391 blocks per core
M4, W4 = 3 * BLK, 18          # G4: virtual rows per block, value width
M2, W2 = 2 * BLK, 8           # G2: virtual rows per block, value width
RC = 6.0
CHUNK = 128                   # tiles per compute chunk


# --------------------------------------------------------------------------
# host-side planning (integer topology work only)
# --------------------------------------------------------------------------

def _pack_stream(n_global, nv, feats, dummies):
    """Route items to cores, sort by destination block, pad to tiles.

    n_global: [T] destination atom row; nv: [T] virtual row in [0, M);
    feats: list of [T] float32 arrays (nv is appended as a float feature).
    Returns (arrs [nfeat+1][8, 128, NT], tiles_per_blk [NBLK]).
    """
    core = n_global // APC
    blk = (n_global % APC) // BLK
    key = core * NBLK + blk
    cnt_cb = np.bincount(key, minlength=N_CORES * NBLK).reshape(N_CORES, NBLK)
    tiles = np.maximum(1, -(-cnt_cb.max(axis=0) // P))      # [NBLK], shared
    blk_off = np.zeros(NBLK + 1, np.int64)
    np.cumsum(tiles * P, out=blk_off[1:])
    S = int(blk_off[-1])                                    # slots per core

    order = np.argsort(key, kind="stable")
    cnt_flat = cnt_cb.reshape(-1)
    gstart = np.zeros(N_CORES * NBLK, np.int64)
    np.cumsum(cnt_flat[:-1], out=gstart[1:])
    rank = np.arange(len(key), dtype=np.int64) - np.repeat(gstart, cnt_flat)
    slot = blk_off[blk[order]] + rank
    core_o = core[order]

    NT = S // P
    out = []
    for f, dummy in zip(feats + [nv.astype(np.float32)], dummies + [-1.0]):
        a = np.full((N_CORES, S), dummy, np.float32)
        a[core_o, slot] = f[order]
        out.append(np.ascontiguousarray(
            a.reshape(N_CORES, NT, P).transpose(0, 2, 1)))  # [8, 128, NT]
    return out, tiles


def _plan(inputs):
    an = np.asarray(inputs["atomic_numbers"])
    ei = np.asarray(inputs["edge_index"])
    D_st = np.asarray(inputs["D_st"], np.float32)
    ba = np.asarray(inputs["id3_ba"])
    ca = np.asarray(inputs["id3_ca"])
    cph = np.asarray(inputs["cos_phi"], np.float32)
    imap = np.asarray(inputs["idx_mapping"])
    imap2 = np.asarray(inputs["idx_mapping_g2"])
    src, dst = ei[0], ei[1]

    # ---- G4: integer mask + destination/slot computation
    keep = ba > ca
    ba = ba[keep]; ca = ca[keep]
    n4 = dst[ca]
    p4 = imap[an[dst[ca]], an[src[ba]], an[src[ca]]]
    nv4 = (n4 % APC % BLK) * 3 + p4
    g4_arrs, tiles4 = _pack_stream(
        n4, nv4,
        [D_st[ba], D_st[ca], cph[keep]],
        [1.0, 1.0, 0.0])

    # ---- G2
    n2 = dst
    s2 = imap2[an[dst], an[src]]
    nv2 = (n2 % APC % BLK) * 2 + s2
    g2_arrs, tiles2 = _pack_stream(n2, nv2, [D_st], [1.0])

    # constants from the parameter tables (tables are uniform by construction)
    g2_etas = np.asarray(inputs["G2_params"], np.float32)[0, 0]        # [8]
    etas = np.asarray(inputs["G4_etas"], np.float32)[0, 0, 0]          # [3]
    zetas = np.asarray(inputs["G4_zetas"], np.float32)[0, 0, 0]        # [3]
    lmdas = np.asarray(inputs["G4_lmdas"], np.float32)[0, 0, 0]        # [2]
    assert np.allclose(zetas, [1.0, 2.0, 4.0]), zetas
    assert np.allclose(sorted(lmdas), [-1.0, 1.0]), lmdas

    return dict(
        dba=g4_arrs[0], dca=g4_arrs[1], cph=g4_arrs[2], nv4=g4_arrs[3],
        d2=g2_arrs[0], nv2=g2_arrs[1],
        tiles4=tiles4, tiles2=tiles2,
        g2_etas=g2_etas, etas=etas, zetas=zetas, lmdas=lmdas,
    )


def _assemble(out4_list, out2_list):
    """[8][M4, NBLK*W4] + [8][M2, NBLK*W2] -> [N_ATOMS, 70] reference layout."""
    full = np.empty((N_ATOMS, 70), np.float32)
    for c in range(N_CORES):
        # G2: rows aloc*2+s, cols k -> out col k*2+s
        o2 = out2_list[c].reshape(BLK, 2, NBLK, W2)
        g2 = o2.transpose(2, 0, 3, 1).reshape(NBLK * BLK, 16)[:APC]
        # G4: rows aloc*3+p, cols i*6+l*3+z -> out col 16+((i*2+l)*3+z)*3+p
        o4 = out4_list[c].reshape(BLK, 3, NBLK, 3, 2, 3)
        g4 = o4.transpose(2, 0, 3, 4, 5, 1).reshape(NBLK * BLK, 54)[:APC]
        full[c * APC:(c + 1) * APC, :16] = g2
        full[c * APC:(c + 1) * APC, 16:] = g4
    return full


# --------------------------------------------------------------------------
# numpy model of exactly what the device computes (for validation)
# --------------------------------------------------------------------------

def _numpy_device_model(plan):
    etas, g2_etas, lmdas = plan["etas"], plan["g2_etas"], plan["lmdas"]
    cz = np.array([0.125 * 2.0 ** (1.0 - z) for z in plan["zetas"]], np.float32)
    out4_list, out2_list = [], []
    for c in range(N_CORES):
        dba, dca, cp, nv4 = (plan[k][c].T.reshape(-1) for k in
                             ("dba", "dca", "cph", "nv4"))
        # wait: [128, NT].T.reshape(-1) gives slot order t*128+p? No:
        # arr[p, t]; .T -> [NT, 128] -> flat index t*128+p = slot. ok.
        b2, c2 = dba * dba, dca * dca
        t4 = b2 + c2
        r2 = np.minimum(np.maximum(t4 - 2.0 * dba * dca * cp, 1e-12), 36.45)
        s = r2 + t4
        ub = 1.0 + np.sin(np.pi / 2 - np.pi / RC * dba)
        uc = 1.0 + np.sin(np.pi / 2 - np.pi / RC * dca)
        ur = 1.0 + np.sin(np.pi / 2 - np.pi / RC * np.sqrt(r2))
        cut = ub * uc * ur
        cut = cut * (dba < RC) * (dca < RC) * (r2 < RC * RC)
        e = [np.exp(-et * s) * cut for et in etas]
        xm = 1.0 + lmdas[0] * cp
        xp = 1.0 + lmdas[1] * cp
        pw = [xm, xm * xm, (xm * xm) * (xm * xm),
              xp, xp * xp, (xp * xp) * (xp * xp)]
        v18 = np.stack([e[i] * pw[l * 3 + z] * cz[z]
                        for i in range(3) for l in range(2) for z in range(3)],
                       axis=1)                                   # [S, 18]
        out4 = np.zeros((M4, NBLK * W4), np.float32)
        tiles4 = plan["tiles4"]
        t_of_blk = np.repeat(np.arange(NBLK), tiles4)
        for t in range(len(t_of_blk)):
            b = t_of_blk[t]
            sl = slice(t * P, (t + 1) * P)
            oh = (nv4[sl][:, None] ==
                  np.arange(M4)[None, :]).astype(np.float32)     # [128, M4]
            out4[:, b * W4:(b + 1) * W4] += oh.T @ v18[sl]
        out4_list.append(out4)

        d2, nv2 = (plan[k][c].T.reshape(-1) for k in ("d2", "nv2"))
        h = 0.5 + 0.5 * np.sin(np.pi / 2 - np.pi / RC * d2)
        v8 = np.stack([h * np.exp(-gk * d2 * d2) for gk in g2_etas], axis=1)
        out2 = np.zeros((M2, NBLK * W2), np.float32)
        tiles2 = plan["tiles2"]
        t_of_blk2 = np.repeat(np.arange(NBLK), tiles2)
        for t in range(len(t_of_blk2)):
            b = t_of_blk2[t]
            sl = slice(t * P, (t + 1) * P)
            oh = (nv2[sl][:, None] ==
                  np.arange(M2)[None, :]).astype(np.float32)
            out2[:, b * W2:(b + 1) * W2] += oh.T @ v8[sl]
        out2_list.append(out2)
    return _assemble(out4_list, out2_list)


# --------------------------------------------------------------------------
# Bass/Tile device kernel
# --------------------------------------------------------------------------

def _build_nc(nt4, tiles4, nt2, tiles2, consts):
    import concourse.bacc as bacc
    import concourse.tile as tile
    from concourse import bass, mybir

    f32 = mybir.dt.float32
    bf16 = mybir.dt.bfloat16
    AF = mybir.ActivationFunctionType
    OP = mybir.AluOpType
    etas, g2_etas, zetas, lmdas = (consts["etas"], consts["g2_etas"],
                                   consts["zetas"], consts["lmdas"])
    cz = [float(0.125 * 2.0 ** (1.0 - z)) for z in zetas]

    nc = bacc.Bacc(None, target_bir_lowering=False)
    din = {}
    for nm, ntt in [("dba", nt4), ("dca", nt4), ("cph", nt4), ("nv4", nt4),
                    ("d2", nt2), ("nv2", nt2), ("iota", M4)]:
        din[nm] = nc.dram_tensor(nm, [P, ntt], f32, kind="ExternalInput")
    out4_d = nc.dram_tensor("out4", [M4, NBLK * W4], f32, kind="ExternalOutput")
    out2_d = nc.dram_tensor("out2", [M2, NBLK * W2], f32, kind="ExternalOutput")

    t4_first = np.zeros(NBLK, np.int64)
    np.cumsum(tiles4[:-1], out=t4_first[1:])
    blk_of_t4 = np.repeat(np.arange(NBLK), tiles4)
    t2_first = np.zeros(NBLK, np.int64)
    np.cumsum(tiles2[:-1], out=t2_first[1:])
    blk_of_t2 = np.repeat(np.arange(NBLK), tiles2)

    GB = 8          # blocks per PSUM super-tile

    with tile.TileContext(nc) as tc, ExitStack() as ctx:
        inp = ctx.enter_context(tc.tile_pool(name="inp", bufs=1))
        ful = ctx.enter_context(tc.tile_pool(name="ful", bufs=1))
        scr = ctx.enter_context(tc.tile_pool(name="scr", bufs=2))
        outp = ctx.enter_context(tc.tile_pool(name="outp", bufs=1))
        psp = ctx.enter_context(tc.tile_pool(name="psum", bufs=2, space="PSUM"))

        consts_sb = {}

        def const(v):
            v = float(v)
            if v not in consts_sb:
                tl = inp.tile([P, 1], f32, tag="const%r" % v,
                              name="c%d" % len(consts_sb))
                nc.vector.memset(tl[:], v)
                consts_sb[v] = tl[:]
            return consts_sb[v]

        V, A = nc.vector, nc.scalar

        sb = {}
        for nm, ntt in [("dba", nt4), ("dca", nt4), ("cph", nt4), ("nv4", nt4),
                        ("d2", nt2), ("nv2", nt2), ("iota", M4)]:
            sb[nm] = inp.tile([P, ntt], f32, tag=nm, name="sb_" + nm)
            nc.sync.dma_start(out=sb[nm][:], in_=din[nm][:])


        def full(name, w=nt4, dt=None):
            return ful.tile([P, w], dt or f32, tag=name, name="f_" + name)

        # bf16 casts of the compare operands
        iotab = full("iotab", M4, bf16)
        V.tensor_copy(out=iotab[:], in_=sb["iota"][:])
        nv4b = full("nv4b", nt4, bf16)
        V.tensor_copy(out=nv4b[:], in_=sb["nv4"][:])
        nv2b = full("nv2b", nt2, bf16)
        V.tensor_copy(out=nv2b[:], in_=sb["nv2"][:])

        dba, dca, cph = sb["dba"][:], sb["dca"][:], sb["cph"][:]
        # ---- phase 1: full-width DVE (G4 geometry) ----
        b2 = full("b2")
        V.tensor_tensor(out=b2[:], in0=dba, in1=dba, op=OP.mult)
        t4 = full("t4")
        V.tensor_tensor(out=t4[:], in0=dca, in1=dca, op=OP.mult)
        V.tensor_tensor(out=t4[:], in0=t4[:], in1=b2[:], op=OP.add)
        bc = full("b2")         # reuse b2 slot; Tile serializes via WAR
        V.tensor_tensor(out=bc[:], in0=dba, in1=dca, op=OP.mult)
        V.tensor_tensor(out=bc[:], in0=bc[:], in1=cph, op=OP.mult)
        r2 = full("r2")
        V.scalar_tensor_tensor(out=r2[:], in0=bc[:], scalar=-2.0,
                               in1=t4[:], op0=OP.mult, op1=OP.add)
        V.tensor_scalar(out=r2[:], in0=r2[:], scalar1=1e-12,
                        scalar2=36.45, op0=OP.max, op1=OP.min)
        s = full("s")
        V.tensor_tensor(out=s[:], in0=r2[:], in1=t4[:], op=OP.add)
        q = full("q", nt2)
        V.tensor_tensor(out=q[:], in0=sb["d2"][:], in1=sb["d2"][:], op=OP.mult)

        # ---- phase 2: grouped ACT (one stream order: Sqrt, Sin*4, Exp*11) --
        hpi, mpio6 = const(math.pi / 2), const(-math.pi / RC)
        e = [full("e%d" % i) for i in range(3)]
        for i in range(3):
            A.activation(out=e[i][:], in_=s[:], func=AF.Exp,
                         scale=const(-float(etas[i])))
        ge = [ful.tile([P, nt2], f32, tag="ge%d" % (k % 2), name="ge%d" % k)
              for k in range(8)]
        for k in range(8):
            A.activation(out=ge[k][:], in_=q[:], func=AF.Exp,
                         scale=const(-float(g2_etas[k])))
        rt = full("s")          # s is consumed by the exps; reuse its slot
        A.activation(out=rt[:], in_=r2[:], func=AF.Sqrt)
        ub, uc, ur = full("ub"), full("uc"), full("ur")
        A.activation(out=ub[:], in_=dba, func=AF.Sin, bias=hpi, scale=mpio6)
        A.activation(out=uc[:], in_=dca, func=AF.Sin, bias=hpi, scale=mpio6)
        A.activation(out=ur[:], in_=rt[:], func=AF.Sin, bias=hpi, scale=mpio6)
        h = full("h", nt2)
        A.activation(out=h[:], in_=sb["d2"][:], func=AF.Sin,
                     bias=hpi, scale=mpio6)

        # ---- phase 2b: full-width DVE combine ----
        h2 = full("h2", nt2)
        V.tensor_scalar(out=h2[:], in0=h[:], scalar1=0.5, scalar2=0.5,
                        op0=OP.mult, op1=OP.add)
        v8 = full("v8", nt2 * W2, bf16)
        for k in range(8):
            V.tensor_tensor(out=v8[:, k * nt2:(k + 1) * nt2],
                            in0=ge[k][:], in1=h2[:], op=OP.mult)
        v8r = v8[:].rearrange("p (k t) -> p k t", t=nt2)
        cut = full("cut")
        V.tensor_scalar(out=cut[:], in0=uc[:], scalar1=1.0, scalar2=None,
                        op0=OP.add)
        V.scalar_tensor_tensor(out=cut[:], in0=ub[:], scalar=1.0,
                               in1=cut[:], op0=OP.add, op1=OP.mult)
        V.scalar_tensor_tensor(out=cut[:], in0=ur[:], scalar=1.0,
                               in1=cut[:], op0=OP.add, op1=OP.mult)
        V.scalar_tensor_tensor(out=cut[:], in0=dba, scalar=RC,
                               in1=cut[:], op0=OP.is_lt, op1=OP.mult)
        V.scalar_tensor_tensor(out=cut[:], in0=dca, scalar=RC,
                               in1=cut[:], op0=OP.is_lt, op1=OP.mult)
        V.scalar_tensor_tensor(out=cut[:], in0=r2[:], scalar=RC * RC,
                               in1=cut[:], op0=OP.is_lt, op1=OP.mult)
        av = [full("av%d" % i, nt4, bf16) for i in range(3)]
        for i in range(3):
            V.tensor_tensor(out=av[i][:], in0=e[i][:], in1=cut[:], op=OP.mult)

        # ---- phase 3: chunked one-hot + v18 + matmuls + batched copies ----
        def run_family(nt, blk_of_t, t_first, tiles, M, W, out_d, mk_chunk):
            psum_cur = [None]
            pshape = [M, GB * W]
            for c0 in range(0, nt, CHUNK):
                cw = min(CHUNK, nt - c0)
                oh, rhs_of = mk_chunk(c0, cw)
                for i in range(cw):
                    tg = c0 + i
                    b = blk_of_t[tg]
                    first = tg == t_first[b]
                    last = tg == t_first[b] + tiles[b] - 1
                    g = b % GB
                    if first and g == 0:
                        psum_cur[0] = psp.tile(pshape, f32, tag="ps%d" % M,
                                               space="PSUM",
                                               name="ps%d_%d" % (M, b))
                    nc.tensor.matmul(
                        out=psum_cur[0][:, g * W:(g + 1) * W],
                        lhsT=oh[:, i * M:(i + 1) * M],
                        rhs=rhs_of(i),
                        start=first, stop=last, skip_group_check=True)
                    if last and (g == GB - 1 or b == NBLK - 1):
                        b0 = b - g
                        cpt = outp.tile([M, GB * W], f32, tag="cp%d" % M,
                                        bufs=3, name="cp%d_%d" % (M, b))
                        A.activation(out=cpt[:, :(g + 1) * W],
                                     in_=psum_cur[0][:, :(g + 1) * W],
                                     func=AF.Copy)
                        nc.sync.dma_start(
                            out=out_d[:, b0 * W:(b + 1) * W],
                            in_=cpt[:, :(g + 1) * W])

        def g4_chunk(c0, cw):
            sl = slice(c0, c0 + cw)
            cp = cph[:, sl]
            pw = {k: scr.tile([P, CHUNK], f32, tag=k, name="p_" + k)[:, :cw]
                  for k in ("xm", "xp", "xm2", "xp2", "xm4", "xp4")}
            V.tensor_scalar(out=pw["xm"], in0=cp, scalar1=float(lmdas[0]),
                            scalar2=1.0, op0=OP.mult, op1=OP.add)
            V.tensor_scalar(out=pw["xp"], in0=cp, scalar1=float(lmdas[1]),
                            scalar2=1.0, op0=OP.mult, op1=OP.add)
            V.tensor_tensor(out=pw["xm2"], in0=pw["xm"], in1=pw["xm"], op=OP.mult)
            V.tensor_tensor(out=pw["xp2"], in0=pw["xp"], in1=pw["xp"], op=OP.mult)
            V.tensor_tensor(out=pw["xm4"], in0=pw["xm2"], in1=pw["xm2"], op=OP.mult)
            V.tensor_tensor(out=pw["xp4"], in0=pw["xp2"], in1=pw["xp2"], op=OP.mult)
            # pwc planes: [128, 6, C], plane (l*3+z) = pw_lz * cz_z
            pwc = scr.tile([P, 6 * CHUNK], bf16, tag="pwc", name="pwc")
            pws = [pw["xm"], pw["xm2"], pw["xm4"], pw["xp"], pw["xp2"], pw["xp4"]]
            for l in range(2):
                for z in range(3):
                    V.tensor_scalar(
                        out=pwc[:, (l * 3 + z) * CHUNK:(l * 3 + z) * CHUNK + cw],
                        in0=pws[l * 3 + z], scalar1=cz[z], scalar2=None,
                        op0=OP.mult)
            # v18 planes: [128, 18, CHUNK]; group i = av_i (bcast) * pwc
            v18 = scr.tile([P, W4 * CHUNK], bf16, tag="v18", name="v18")
            pwcv = pwc[:].rearrange("p (k c) -> p k c", c=CHUNK)[:, :, :cw]
            for i in range(3):
                grp = v18[:, i * 6 * CHUNK:(i + 1) * 6 * CHUNK]
                grpv = grp.rearrange("p (k c) -> p k c", c=CHUNK)[:, :, :cw]
                V.tensor_tensor(
                    out=grpv,
                    in0=av[i][:, sl][:, None, :].to_broadcast([P, 6, cw]),
                    in1=pwcv, op=OP.mult)
            v18r = v18[:].rearrange("p (k c) -> p k c", c=CHUNK)
            oh = scr.tile([P, CHUNK * M4], bf16, tag="oh", name="oh4")
            ohv = oh[:].rearrange("p (c m) -> p c m", m=M4)
            V.tensor_tensor(
                out=ohv[:, :cw, :],
                in0=nv4b[:, sl][:, :, None].to_broadcast([P, cw, M4]),
                in1=iotab[:, None, :].to_broadcast([P, cw, M4]),
                op=OP.is_equal)
            return oh, lambda i: v18r[:, :, i]

        def g2_chunk(c0, cw):
            sl = slice(c0, c0 + cw)
            oh = scr.tile([P, CHUNK * M2], bf16, tag="oh2", name="oh2")
            ohv = oh[:].rearrange("p (c m) -> p c m", m=M2)
            V.tensor_tensor(
                out=ohv[:, :cw, :],
                in0=nv2b[:, sl][:, :, None].to_broadcast([P, cw, M2]),
                in1=iotab[:, None, :M2].to_broadcast([P, cw, M2]),
                op=OP.is_equal)
            return oh, lambda i: v8r[:, :, c0 + i]

        run_family(nt4, blk_of_t4, t4_first, tiles4, M4, W4, out4_d[:],
                   g4_chunk)
        run_family(nt2, blk_of_t2, t2_first, tiles2, M2, W2, out2_d[:],
                   g2_chunk)
    nc.finalize()
    return nc


# --------------------------------------------------------------------------
# entry point
# --------------------------------------------------------------------------

def kernel(**inputs):
    from concourse.bass_utils import run_bass_kernel_spmd

    plan = _plan(inputs)
    nt4 = plan["dba"].shape[2]
    nt2 = plan["d2"].shape[2]
    consts = {k: plan[k] for k in ("etas", "g2_etas", "zetas", "lmdas")}
    nc = _build_nc(nt4, plan["tiles4"], nt2, plan["tiles2"], consts)

    iota = np.broadcast_to(np.arange(M4, dtype=np.float32), (P, M4)).copy()
    in_maps = []
    for c in range(N_CORES):
        in_maps.append(dict(
            dba=plan["dba"][c], dca=plan["dca"][c], cph=plan["cph"][c],
            nv4=plan["nv4"][c], d2=plan["d2"][c], nv2=plan["nv2"][c],
            iota=iota))
    res = run_bass_kernel_spmd(nc, in_maps, core_ids=list(range(N_CORES)))
    out4_list = [r["out4"] for r in res.results]
    out2_list = [r["out2"] for r in res.results]
    return _assemble(out4_list, out2_list)



# revision 5
# speedup vs baseline: 1.3235x; 1.3235x over previous
"""ACSF descriptor kernel for 8 Trainium2 NeuronCores (Bass/Tile), v2.

Scheme
------
Output rows (atoms) are sharded across the 8 cores (6250 atoms each).
The host does integer-only topology preprocessing:
  * drops triplets failing the integer mask id3_ba > id3_ca,
  * computes each item's destination atom row and species column-slot
    (from the idx_mapping tables),
  * routes items to the owning core, sorts them by 16-atom destination
    block, pads each block's item list to whole 128-item tiles (shared
    tile counts across cores so one SPMD program serves all 8), and
  * emits the per-tile one-hot scatter matrices as fp8 bytes (0/1 are
    exact in fp8e4m3) which are DMA'd to the device.

The device does all floating-point math (cutoffs, exps, powers) and the
scatter-add.  Scatter-add: per 128-item tile, TensorE computes
values^T @ one_hot -> PSUM [W, M] where W is the per-item value width
(18 for G4, 8 for G2) and M = 16 atoms x slots (48 / 32).  The narrow
values matrix is the stationary operand (cheap LDWEIGHTS), the one-hot
streams.  Four consecutive atom blocks are stacked into the four
32-partition PSUM column groups (tile_position col-tiling) so their
matmuls execute concurrently in the PE array and the PSUM->SBUF copy
moves 4 blocks at once at full partition width.
"""

import math
from contextlib import ExitStack

import numpy as np

P = 128          # partitions / items per tile
N_ATOMS = 50000
N_CORES = 8
APC = N_ATOMS // N_CORES      # atoms per core (6250)
BLK = 16                      # atoms per block
NBLK = (APC + BLK - 1) // BLK # 391 blocks per core
M4, W4 = 3 * BLK, 18          # G4: one-hot width per block, value width
M2, W2 = 2 * BLK, 8           # G2
RC = 6.0
QG = 4                        # blocks per quad (PSUM col groups)
NQ = (NBLK + QG - 1) // QG    # 98 quads
SQ = 8                        # quads per PSUM supertile
NST = (NQ + SQ - 1) // SQ     # 13 supertiles


# --------------------------------------------------------------------------
# host-side planning (integer topology work only)
# --------------------------------------------------------------------------

def _pack_stream(n_global, nv, feats, dummies):
    """Route items to cores, sort by destination block, pad to tiles.

    n_global: [T] destination atom row; nv: [T] virtual row in [0, M);
    feats: list of [T] float32 arrays (nv is appended as a float feature).
    Returns (arrs [nfeat+1][8, 128, NT], tiles_per_blk [NBLK]).
    """
    core = n_global // APC
    blk = (n_global % APC) // BLK
    key = core * NBLK + blk
    cnt_cb = np.bincount(key, minlength=N_CORES * NBLK).reshape(N_CORES, NBLK)
    tiles = np.maximum(1, -(-cnt_cb.max(axis=0) // P))      # [NBLK], shared
    blk_off = np.zeros(NBLK + 1, np.int64)
    np.cumsum(tiles * P, out=blk_off[1:])
    S = int(blk_off[-1])                                    # slots per core

    order = np.argsort(key, kind="stable")
    cnt_flat = cnt_cb.reshape(-1)
    gstart = np.zeros(N_CORES * NBLK, np.int64)
    np.cumsum(cnt_flat[:-1], out=gstart[1:])
    rank = np.arange(len(key), dtype=np.int64) - np.repeat(gstart, cnt_flat)
    slot = blk_off[blk[order]] + rank
    core_o = core[order]

    NT = S // P
    out = []
    for f, dummy in zip(feats + [nv.astype(np.float32)], dummies + [-1.0]):
        a = np.full((N_CORES, S), dummy, np.float32)
        a[core_o, slot] = f[order]
        out.append(np.ascontiguousarray(
            a.reshape(N_CORES, NT, P).transpose(0, 2, 1)))  # [8, 128, NT]
    return out, tiles


def _one_hot_fp8(nv, M):
    """nv: [8, 128, NT] float32 virtual rows (-1 = padding) ->
    uint8 fp8e4m3 one-hot [8, 128, NT*M] (0x38 == 1.0)."""
    C, Pp, NT = nv.shape
    oh = np.zeros((C, Pp, NT * M), np.uint8)
    c, p, t = np.nonzero(nv >= 0.0)
    m = nv[c, p, t].astype(np.int64)
    oh[c, p, t * M + m] = 0x38
    return oh


def _plan(inputs):
    an = np.asarray(inputs["atomic_numbers"])
    ei = np.asarray(inputs["edge_index"])
    D_st = np.asarray(inputs["D_st"], np.float32)
    ba = np.asarray(inputs["id3_ba"])
    ca = np.asarray(inputs["id3_ca"])
    cph = np.asarray(inputs["cos_phi"], np.float32)
    imap = np.asarray(inputs["idx_mapping"])
    imap2 = np.asarray(inputs["idx_mapping_g2"])
    src, dst = ei[0], ei[1]

    # ---- G4: integer mask + destination/slot computation
    keep = ba > ca
    ba = ba[keep]; ca = ca[keep]
    n4 = dst[ca]
    p4 = imap[an[dst[ca]], an[src[ba]], an[src[ca]]]
    nv4 = (n4 % APC % BLK) * 3 + p4
    g4_arrs, tiles4 = _pack_stream(
        n4, nv4,
        [D_st[ba], D_st[ca], cph[keep]],
        [1.0, 1.0, 0.0])

    # ---- G2
    n2 = dst
    s2 = imap2[an[dst], an[src]]
    nv2 = (n2 % APC % BLK) * 2 + s2
    g2_arrs, tiles2 = _pack_stream(n2, nv2, [D_st], [1.0])

    # constants from the parameter tables (tables are uniform by construction)
    g2_etas = np.asarray(inputs["G2_params"], np.float32)[0, 0]        # [8]
    etas = np.asarray(inputs["G4_etas"], np.float32)[0, 0, 0]          # [3]
    zetas = np.asarray(inputs["G4_zetas"], np.float32)[0, 0, 0]        # [3]
    lmdas = np.asarray(inputs["G4_lmdas"], np.float32)[0, 0, 0]        # [2]
    assert np.allclose(zetas, [1.0, 2.0, 4.0]), zetas
    assert np.allclose(sorted(lmdas), [-1.0, 1.0]), lmdas

    return dict(
        dba=g4_arrs[0], dca=g4_arrs[1], cph=g4_arrs[2],
        oh4=_one_hot_fp8(g4_arrs[3], M4),
        d2=g2_arrs[0],
        oh2=_one_hot_fp8(g2_arrs[1], M2),
        tiles4=tiles4, tiles2=tiles2,
        g2_etas=g2_etas, etas=etas, zetas=zetas, lmdas=lmdas,
    )


def _assemble(out4_list, out2_list):
    """[8][128, NQ*W4... cols] + [8][128, NQ*M2] -> [N_ATOMS, 70]."""
    full = np.empty((N_ATOMS, 70), np.float32)
    pad = NQ * QG * BLK                      # 6272 padded atoms per core
    for c in range(N_CORES):
        # G4: partition 32g+j (j<18 = value plane), col q*48 + aloc*3 + p
        o4 = out4_list[c].reshape(QG, 32, NQ, BLK, 3)[:, :W4]
        g4 = o4.transpose(2, 0, 3, 1, 4).reshape(pad, 54)[:APC]
        # G2: partition 32g+k (k<8), col q*32 + aloc*2 + s
        o2 = out2_list[c].reshape(QG, 32, NQ, BLK, 2)[:, :W2]
        g2 = o2.transpose(2, 0, 3, 1, 4).reshape(pad, 16)[:APC]
        full[c * APC:(c + 1) * APC, :16] = g2
        full[c * APC:(c + 1) * APC, 16:] = g4
    return full


# --------------------------------------------------------------------------
# Bass/Tile device kernel
# --------------------------------------------------------------------------

def _build_nc(nt4, tiles4, nt2, tiles2, consts):
    import concourse.bacc as bacc
    import concourse.tile as tile
    from concourse import bass, mybir

    f32 = mybir.dt.float32
    bf16 = mybir.dt.bfloat16
    f8 = mybir.dt.float8e4
    AF = mybir.ActivationFunctionType
    OP = mybir.AluOpType
    etas, g2_etas, zetas, lmdas = (consts["etas"], consts["g2_etas"],
                                   consts["zetas"], consts["lmdas"])
    cz = [float(0.125 * 2.0 ** (1.0 - z)) for z in zetas]

    nc = bacc.Bacc(None, target_bir_lowering=False)
    din = {}
    for nm, ntt in [("dba", nt4), ("dca", nt4), ("cph", nt4), ("d2", nt2)]:
        din[nm] = nc.dram_tensor(nm, [P, ntt], f32, kind="ExternalInput")
    oh4_d = nc.dram_tensor("oh4", [P, nt4 * M4], f8, kind="ExternalInput")
    oh2_d = nc.dram_tensor("oh2", [P, nt2 * M2], f8, kind="ExternalInput")
    out4_d = nc.dram_tensor("out4", [P, NQ * M4], f32, kind="ExternalOutput")
    out2_d = nc.dram_tensor("out2", [P, NQ * M2], f32, kind="ExternalOutput")

    CH4 = -(-nt4 // 12)           # one-hot DMA chunk (tiles)
    CH2 = -(-nt2 // 8)

    with tile.TileContext(nc) as tc, ExitStack() as ctx:
        inp = ctx.enter_context(tc.tile_pool(name="inp", bufs=1))
        ful = ctx.enter_context(tc.tile_pool(name="ful", bufs=1))
        scr = ctx.enter_context(tc.tile_pool(name="scr", bufs=1))
        ohp = ctx.enter_context(tc.tile_pool(name="ohp", bufs=2))
        outp = ctx.enter_context(tc.tile_pool(name="outp", bufs=2))
        psp = ctx.enter_context(tc.tile_pool(name="psum", bufs=2, space="PSUM"))

        consts_sb = {}

        def const(v):
            v = float(v)
            if v not in consts_sb:
                tl = inp.tile([P, 1], f32, tag="const%r" % v,
                              name="c%d" % len(consts_sb))
                nc.vector.memset(tl[:], v)
                consts_sb[v] = tl[:]
            return consts_sb[v]

        V, A = nc.vector, nc.scalar

        sb = {}
        for nm, ntt in [("dba", nt4), ("dca", nt4), ("cph", nt4), ("d2", nt2)]:
            sb[nm] = inp.tile([P, ntt], f32, tag=nm, name="sb_" + nm)
            nc.sync.dma_start(out=sb[nm][:], in_=din[nm][:])

        def full(name, w=nt4, dt=None):
            return ful.tile([P, w], dt or f32, tag=name, name="f_" + name)

        dba, dca, cph = sb["dba"][:], sb["dca"][:], sb["cph"][:]
        # ---- G4 geometry (fp32, DVE) ----
        b2 = full("b2")
        V.tensor_tensor(out=b2[:], in0=dba, in1=dba, op=OP.mult)
        t4 = full("t4")
        V.tensor_tensor(out=t4[:], in0=dca, in1=dca, op=OP.mult)
        V.tensor_tensor(out=t4[:], in0=t4[:], in1=b2[:], op=OP.add)
        bc = full("b2")         # reuse b2 slot; Tile serializes via WAR
        V.tensor_tensor(out=bc[:], in0=dba, in1=dca, op=OP.mult)
        V.tensor_tensor(out=bc[:], in0=bc[:], in1=cph, op=OP.mult)
        r2 = full("r2")
        V.scalar_tensor_tensor(out=r2[:], in0=bc[:], scalar=-2.0,
                               in1=t4[:], op0=OP.mult, op1=OP.add)
        V.tensor_scalar(out=r2[:], in0=r2[:], scalar1=1e-12,
                        scalar2=36.45, op0=OP.max, op1=OP.min)
        s = full("s")
        V.tensor_tensor(out=s[:], in0=r2[:], in1=t4[:], op=OP.add)
        q = full("b2", nt2)     # geometry temps are dead; reuse slots
        V.tensor_tensor(out=q[:], in0=sb["d2"][:], in1=sb["d2"][:], op=OP.mult)

        # ---- activations (ScalarE): Sqrt, Sin x4, Exp x11 ----
        hpi, mpio6 = const(math.pi / 2), const(-math.pi / RC)
        rt = full("t4")
        A.activation(out=rt[:], in_=r2[:], func=AF.Sqrt)
        ub = full("ub", nt4, bf16)
        uc = full("uc", nt4, bf16)
        ur = full("ur", nt4, bf16)
        A.activation(out=ub[:], in_=dba, func=AF.Sin, bias=hpi, scale=mpio6)
        A.activation(out=uc[:], in_=dca, func=AF.Sin, bias=hpi, scale=mpio6)
        A.activation(out=ur[:], in_=rt[:], func=AF.Sin, bias=hpi, scale=mpio6)
        h = full("h", nt2, bf16)
        A.activation(out=h[:], in_=sb["d2"][:], func=AF.Sin,
                     bias=hpi, scale=mpio6)
        e = [full("e%d" % i, nt4, bf16) for i in range(3)]
        for i in range(3):
            A.activation(out=e[i][:], in_=s[:], func=AF.Exp,
                         scale=const(-float(etas[i])))

        # ---- cutoff (fp32 chain on DVE; bf16 inputs upcast) ----
        cut = full("s")
        V.tensor_scalar(out=cut[:], in0=uc[:], scalar1=1.0, scalar2=None,
                        op0=OP.add)
        V.scalar_tensor_tensor(out=cut[:], in0=ub[:], scalar=1.0,
                               in1=cut[:], op0=OP.add, op1=OP.mult)
        V.scalar_tensor_tensor(out=cut[:], in0=ur[:], scalar=1.0,
                               in1=cut[:], op0=OP.add, op1=OP.mult)
        V.scalar_tensor_tensor(out=cut[:], in0=dba, scalar=RC,
                               in1=cut[:], op0=OP.is_lt, op1=OP.mult)
        V.scalar_tensor_tensor(out=cut[:], in0=dca, scalar=RC,
                               in1=cut[:], op0=OP.is_lt, op1=OP.mult)
        cutb = full("cutb", nt4, bf16)
        V.scalar_tensor_tensor(out=cutb[:], in0=r2[:], scalar=RC * RC,
                               in1=cut[:], op0=OP.is_lt, op1=OP.mult)
        av = [full(nm, nt4, bf16) for nm in ("ub", "uc", "ur")]
        for i in range(3):
            V.tensor_tensor(out=av[i][:], in0=e[i][:], in1=cutb[:], op=OP.mult)

        # ---- G2 values, plane-major [P, 8, nt2] bf16 ----
        h2 = full("h2", nt2, bf16)
        V.tensor_scalar(out=h2[:], in0=h[:], scalar1=0.5, scalar2=0.5,
                        op0=OP.mult, op1=OP.add)
        v8 = full("v8", W2 * nt2, bf16)
        ge = [ful.tile([P, nt2], bf16, tag="ge%d" % (k % 2), name="ge%d" % k)
              for k in range(W2)]
        for k in range(W2):
            A.activation(out=ge[k][:], in_=q[:], func=AF.Exp,
                         scale=const(-float(g2_etas[k])))
            V.tensor_tensor(out=v8[:, k * nt2:(k + 1) * nt2],
                            in0=ge[k][:], in1=h2[:], op=OP.mult)
        v8r = v8[:].rearrange("p (k t) -> p k t", t=nt2)

        # ---- G4 values: pw chain (2 half passes), pwc, v18 plane-major ----
        pwc = full("pwc", 6 * nt4, bf16)
        HW = -(-nt4 // 2)
        for hp in range(2):
            sl = slice(hp * HW, min((hp + 1) * HW, nt4))
            w = sl.stop - sl.start
            cp = cph[:, sl]
            pw = {k: scr.tile([P, HW], f32, tag=k, name="pw_%s%d" % (k, hp))
                  for k in ("xm", "xp", "xm2", "xp2", "xm4", "xp4")}
            V.tensor_scalar(out=pw["xm"][:, :w], in0=cp,
                            scalar1=float(lmdas[0]), scalar2=1.0,
                            op0=OP.mult, op1=OP.add)
            V.tensor_scalar(out=pw["xp"][:, :w], in0=cp,
                            scalar1=float(lmdas[1]), scalar2=1.0,
                            op0=OP.mult, op1=OP.add)
            V.tensor_tensor(out=pw["xm2"][:, :w], in0=pw["xm"][:, :w],
                            in1=pw["xm"][:, :w], op=OP.mult)
            V.tensor_tensor(out=pw["xp2"][:, :w], in0=pw["xp"][:, :w],
                            in1=pw["xp"][:, :w], op=OP.mult)
            V.tensor_tensor(out=pw["xm4"][:, :w], in0=pw["xm2"][:, :w],
                            in1=pw["xm2"][:, :w], op=OP.mult)
            V.tensor_tensor(out=pw["xp4"][:, :w], in0=pw["xp2"][:, :w],
                            in1=pw["xp2"][:, :w], op=OP.mult)
            pws = [pw["xm"], pw["xm2"], pw["xm4"],
                   pw["xp"], pw["xp2"], pw["xp4"]]
            for lz in range(6):
                V.tensor_scalar(
                    out=pwc[:, lz * nt4 + sl.start:lz * nt4 + sl.stop],
                    in0=pws[lz][:, :w], scalar1=cz[lz % 3], scalar2=None,
                    op0=OP.mult)
        # v18 planes ordered j = i*6 + l*3 + z  (== reference (i*2+l)*3+z)
        v18 = full("v18", W4 * nt4, bf16)
        pwcv = pwc[:].rearrange("p (k t) -> p k t", t=nt4)
        for i in range(3):
            grp = v18[:, i * 6 * nt4:(i + 1) * 6 * nt4]
            grpv = grp.rearrange("p (k t) -> p k t", t=nt4)
            V.tensor_tensor(
                out=grpv,
                in0=av[i][:, None, :].to_broadcast([P, 6, nt4]),
                in1=pwcv, op=OP.mult)
        v18r = v18[:].rearrange("p (k t) -> p k t", t=nt4)

        # ---- scatter: col-tiled matmuls + PSUM supertile copies ----
        def run_family(nt, tiles, M, W, vr, oh_d, out_d, ch):
            t_first = np.zeros(NBLK, np.int64)
            np.cumsum(tiles[:-1], out=t_first[1:])
            oh_tiles = {}                # two most recent chunks stay alive

            def oh_rhs(tg):
                cidx = tg // ch
                if cidx not in oh_tiles:
                    for k in [k for k in oh_tiles if k < cidx - 1]:
                        del oh_tiles[k]
                    c0 = cidx * ch
                    cw = min(ch, nt - c0)
                    t = ohp.tile([P, ch * M], mybir.dt.float8e4,
                                 tag="oh%d" % M, name="oh%d_%d" % (M, cidx))
                    nc.sync.dma_start(out=t[:, :cw * M],
                                      in_=oh_d[:, c0 * M:(c0 + cw) * M])
                    oh_tiles[cidx] = t
                off = (tg - cidx * ch) * M
                return oh_tiles[cidx][:, off:off + M]

            for st in range(NST):
                q0, q1 = st * SQ, min((st + 1) * SQ, NQ)
                wcols = (q1 - q0) * M
                ps = psp.tile([P, SQ * M], mybir.dt.float32, tag="ps%d" % M,
                              space="PSUM", name="ps%d_%d" % (M, st))
                for qq, qd in enumerate(range(q0, q1)):
                    blks = range(qd * QG, min((qd + 1) * QG, NBLK))
                    jmax = max(tiles[b] for b in blks)
                    for j in range(jmax):
                        for g, b in enumerate(blks):
                            if j >= tiles[b]:
                                continue
                            tg = int(t_first[b] + j)
                            nc.tensor.matmul(
                                out=ps[32 * g:32 * g + W,
                                       qq * M:(qq + 1) * M],
                                lhsT=vr[:, :, tg],
                                rhs=oh_rhs(tg),
                                start=(j == 0), stop=(j == tiles[b] - 1),
                                skip_group_check=True,
                                tile_position=(0, 32 * g))
                cpt = outp.tile([P, SQ * M], mybir.dt.float32, tag="cp%d" % M,
                                name="cp%d_%d" % (M, st))
                A.activation(out=cpt[:, :wcols], in_=ps[:, :wcols],
                             func=AF.Copy)
                nc.sync.dma_start(out=out_d[:, q0 * M:q1 * M],
                                  in_=cpt[:, :wcols])

        run_family(nt4, tiles4, M4, W4, v18r, oh4_d[:], out4_d[:], CH4)
        run_family(nt2, tiles2, M2, W2, v8r, oh2_d[:], out2_d[:], CH2)
    nc.finalize()
    return nc


# --------------------------------------------------------------------------
# entry point
# --------------------------------------------------------------------------

def _run(inputs, trace=False):
    import ml_dtypes
    from concourse.bass_utils import run_bass_kernel_spmd

    plan = _plan(inputs)
    nt4 = plan["dba"].shape[2]
    nt2 = plan["d2"].shape[2]
    consts = {k: plan[k] for k in ("etas", "g2_etas", "zetas", "lmdas")}
    nc = _build_nc(nt4, plan["tiles4"], nt2, plan["tiles2"], consts)

    in_maps = []
    for c in range(N_CORES):
        in_maps.append(dict(
            dba=plan["dba"][c], dca=plan["dca"][c], cph=plan["cph"][c],
            d2=plan["d2"][c],
            oh4=plan["oh4"][c].view(ml_dtypes.float8_e4m3fn),
            oh2=plan["oh2"][c].view(ml_dtypes.float8_e4m3fn)))
    res = run_bass_kernel_spmd(nc, in_maps, core_ids=list(range(N_CORES)),
                               trace=trace)
    out = _assemble([r["out4"] for r in res.results],
                    [r["out2"] for r in res.results])
    return out, res


def kernel(**inputs):
    return _run(inputs)[0]
